# revision 10
# baseline (speedup 1.0000x reference)
"""Trainium2 Bass kernel for a dense-transformer attention block.

Reference semantics (T=2048, D=2048, 16 heads, d_h=128):
    h = RMSNorm(x) * ln_w
    q,k,v = h @ W{q,k,v}.T  -> (n_h, T, d_h);  RoPE(q, k)
    att = softmax(causal(q k^T / sqrt(d_h))) @ v
    out = x + att @ Wo.T          (attention_mask is all-ones per spec)

Distribution: head-parallel over 8 cores (2 heads/core).  Each core:
  phase 1  QKV projections for its heads (bf16 matmuls, contract over d_model);
           RMSNorm row sums computed OFF the PE: x (row-major) streams through
           ScalarE Square with free-dim accumulate -> per-row sums, then a
           short [128,4] reciprocal+sqrt chain; r enters via r-scaled RoPE
           tables (q,k) and per-row scaling (v); ln_w folded into the weights.
           rotate_half runs on the PE as a constant permutation matmul.
  phase 2  per-head causal attention with scores computed TRANSPOSED
           (S^T[j,i]) so no transposes are needed anywhere; softmax row-sums
           accumulate on the PE via a ones-vector matmul; exp on ScalarE.
  phase 3  per-block AllGather of att^T rows in fp8-e3m4, BUNDLED across the
           two heads (4 collectives total: the CC core runs collectives
           serially at ~24us each, so fewer/larger beats many/small).
  phase 4  output projection column-shard, weight-stationary; emitted
           interleaved (block B-1's projection right after block B's
           attention) so the PE fills the tail while the last AllGather is
           in flight:  out^T[:, cols] = sum_k WoT-chunk.T @ attT-chunk + x.
Host assembles out = concat(out_colsT.T, axis=1).

fp8 notes from numerics simulation (err_sim.py): the e3m4 att wire is the
dominant error term (9.9e-3 of the 2e-2 budget); e4m3 wire alone hits
1.86e-2 and e4m3 Wo 3.9e-2, so phase 4 stays bf16.  fp8 probs (for
DoubleRow att@V) are impossible without per-row max subtraction: scores
span [-11, +8.6] with row-maxima as low as -3.2, far beyond fp8 range
under any global exp scale.
"""

import math

import numpy as np

EPS = 1e-5
NEG = -1.0e30

CFG_FULL = dict(T=2048, D=2048, n_cores=8, heads_per_core=2)


# --------------------------------------------------------------------------
# device program
# --------------------------------------------------------------------------
def build_nc(T, D, n_cores, heads_per_core):
    import concourse.mybir as mybir
    import concourse.tile as tile
    from concourse import bacc

    DH = 128                      # head dim (hard-wired into layout)
    P = 128                       # partitions
    NH = heads_per_core
    DL = NH * DH                  # local width (q/k/v columns per core)
    KC = D // P                   # k-chunks over d_model
    TB = T // 512                 # 512-wide t blocks
    NTS = T // P                  # 128-wide t subtiles
    f32 = mybir.dt.float32
    bf16 = mybir.dt.bfloat16
    f8 = mybir.dt.float8e3

    nc = bacc.Bacc("TRN2", target_bir_lowering=False, debug=False,
                   num_devices=n_cores)

    # ---- I/O ----
    xT = nc.dram_tensor("xT", [D, T], bf16, kind="ExternalInput").ap()
    xR = nc.dram_tensor("xR", [T, D], bf16, kind="ExternalInput").ap()
    xct_in = nc.dram_tensor("x_colsT", [DL, T], f32, kind="ExternalInput").ap()
    # weight tensors arrive host-pretiled in SBUF layout [P, KC*DL]
    wq_t = nc.dram_tensor("wq_t", [P, KC * DL], bf16, kind="ExternalInput").ap()
    wk_t = nc.dram_tensor("wk_t", [P, KC * DL], bf16, kind="ExternalInput").ap()
    wv_t = nc.dram_tensor("wv_t", [P, KC * DL], bf16, kind="ExternalInput").ap()
    # wo additionally row-permuted on host to the AllGather chunk order
    wo_t = nc.dram_tensor("wo_t", [P, KC * DL], bf16, kind="ExternalInput").ap()
    cosT = nc.dram_tensor("cosT", [DH, T], f32, kind="ExternalInput").ap()
    sinT = nc.dram_tensor("sinT", [DH, T], f32, kind="ExternalInput").ap()
    rot_t = nc.dram_tensor("rot_t", [DH, DH], bf16, kind="ExternalInput").ap()
    lnw = nc.dram_tensor("ln_w", [D], f32, kind="ExternalInput").ap()
    out_cT = nc.dram_tensor("out_colsT", [DL, T], f32,
                            kind="ExternalOutput").ap()

    Act = mybir.ActivationFunctionType
    Alu = mybir.AluOpType
    inv_sqrt_dh = 1.0 / math.sqrt(DH)

    with tile.TileContext(nc) as tc, \
            tc.tile_pool(name="persist", bufs=1) as persist:
        # ---------------- long-lived tensors ----------------
        Q_sb = persist.tile([P, NH, T], bf16, tag="Q_sb")
        K_sb = persist.tile([P, NH, T], bf16, tag="K_sb")
        V_sb = persist.tile([P, NTS, DL], bf16, tag="V_sb")
        wo_sb = persist.tile([P, KC, DL], bf16, tag="wo_sb")
        rcol_sb = persist.tile([P, NTS], f32, tag="rcol_sb")
        rrow_sb = persist.tile([1, T], f32, tag="rrow_sb")
        ssq_sb = persist.tile([P, NTS], f32, tag="ssq_sb")
        ones_bf = persist.tile([P, 1], bf16, tag="ones_bf")
        masks_sb = persist.tile([P, 4, 512], f32, tag="masks_sb")
        rot_sb = persist.tile([P, DH], bf16, tag="rot_sb")

        nc.gpsimd.dma_start(rot_sb[:], rot_t)
        nc.vector.memset(ones_bf[:], 1.0)
        warm_sb = persist.tile([P, 128], bf16, tag="warm_sb")
        nc.vector.memset(warm_sb[:], 0.0)
        warm_big = persist.tile([P, 512], bf16, tag="warm_big")
        nc.vector.memset(warm_big[:], 0.0)
        nc.gpsimd.memset(masks_sb[:], 0.0)
        for rr_ in range(4):
            # keep (0) where i - j >= 0 with i = 512*B + f, j = 128*J + p,
            # offset r = J - 4*B  ->  f - p - 128 r >= 0
            nc.gpsimd.affine_select(
                out=masks_sb[:, rr_, :], in_=masks_sb[:, rr_, :],
                pattern=[[1, 512]], channel_multiplier=-1, base=-128 * rr_,
                compare_op=Alu.is_ge, fill=NEG)

        with tc.tile_pool(name="dram", bufs=1, space="DRAM") as dram_pool:
            ag_shared = "Shared" if n_cores > 4 else "Local"
            # bundled (both heads) AG buffers, one collective per t-block;
            # gathered rows are source-core-major: chunk kc = 2c + h
            agb_in = [dram_pool.tile([NH * DH, 512], f8, tag=f"agbi{b}",
                                     name=f"agb_in{b}")
                      for b in range(TB)]
            agb_out = [dram_pool.tile([n_cores * NH * DH, 512], f8,
                                      addr_space=ag_shared, tag=f"agbo{b}",
                                      name=f"agb_out{b}")
                       for b in range(TB)]

            # PE warmup: back-to-back dummy matmuls so the HAM clock gate
            # opens during the initial DMA wait
            with tc.tile_pool(name="warm_ps", bufs=1, space="PSUM") as wmps:
                wps = wmps.tile([P, 512], f32, tag="wm")
                for _ in range(16):
                    nc.tensor.matmul(wps[:], warm_sb[:], warm_big[:],
                                     start=True, stop=True)

            from contextlib import ExitStack
            with ExitStack() as stack:
                wpool = stack.enter_context(tc.tile_pool(name="wqkv", bufs=1))
                cspool = stack.enter_context(tc.tile_pool(name="cs_raw", bufs=1))
                xpool = stack.enter_context(tc.tile_pool(name="xk", bufs=1))
                xrpool = stack.enter_context(tc.tile_pool(name="xr", bufs=2))
                sqpool = stack.enter_context(tc.tile_pool(name="sqs", bufs=1))
                tmppool = stack.enter_context(tc.tile_pool(name="tmp1", bufs=4))
                rbcpool = stack.enter_context(tc.tile_pool(name="rbc1", bufs=2))
                ptpool = stack.enter_context(tc.tile_pool(name="pt", bufs=4))
                finpool = stack.enter_context(tc.tile_pool(name="fin", bufs=2))
                agpool = stack.enter_context(tc.tile_pool(name="ag_sb", bufs=20))
                xcpool = stack.enter_context(tc.tile_pool(name="xc", bufs=2))
                opool = stack.enter_context(tc.tile_pool(name="osb", bufs=2))
                qkps = stack.enter_context(
                    tc.tile_pool(name="qk_ps", bufs=1, space="PSUM"))
                vps = stack.enter_context(
                    tc.tile_pool(name="v_ps", bufs=1, space="PSUM"))
                rowps = stack.enter_context(
                    tc.tile_pool(name="row_ps", bufs=1, space="PSUM"))
                stpool = stack.enter_context(
                    tc.tile_pool(name="st_ps", bufs=2, space="PSUM"))
                avpool = stack.enter_context(
                    tc.tile_pool(name="av_ps", bufs=1, space="PSUM"))
                ops = stack.enter_context(
                    tc.tile_pool(name="o_ps", bufs=1, space="PSUM"))
                lnw_sb = wpool.tile([P, KC], f32, tag="lnw")
                nc.sync.dma_start(lnw_sb[:], lnw.rearrange("(kc p) -> p kc", p=P))
                wq_sb = wpool.tile([P, KC, DL], bf16, tag="wq")
                wk_sb = wpool.tile([P, KC, DL], bf16, tag="wk")
                wv_sb = wpool.tile([P, KC, DL], bf16, tag="wv")
                # load order = need order: wq -> x^T chunks -> x rows (block
                # 0 RMS) -> wk -> cos/sin -> wv -> wo
                xk = [xpool.tile([P, T], bf16, tag=f"xk{kc}", name=f"xk{kc}")
                      for kc in range(KC)]
                nc.sync.dma_start(wq_sb[:], wq_t.rearrange("p (kc j) -> p kc j", j=DL))
                for kc in range(KC):
                    nc.sync.dma_start(xk[kc][:], xT[P * kc:P * (kc + 1), :])
                xr_tiles = {}
                for ts in range(4):
                    t_ = xrpool.tile([P, D], bf16, tag="xr", name=f"xr{ts}")
                    nc.sync.dma_start(t_[:], xR[P * ts:P * (ts + 1), :])
                    xr_tiles[ts] = t_
                nc.sync.dma_start(wk_sb[:], wk_t.rearrange("p (kc j) -> p kc j", j=DL))
                # cos/sin tables; r is folded in per block, in place
                cos_r = cspool.tile([P, T], f32, tag="cos")
                sin_r = cspool.tile([P, T], f32, tag="sin")
                nc.sync.dma_start(cos_r[:], cosT)
                nc.sync.dma_start(sin_r[:], sinT)
                nc.sync.dma_start(wv_sb[:], wv_t.rearrange("p (kc j) -> p kc j", j=DL))
                nc.sync.dma_start(wo_sb[:], wo_t.rearrange("p (kc j) -> p kc j", j=DL))
                # fold ln_w into the projection weights (free-dim broadcast,
                # quarter granularity so the first matmuls unblock early)
                qn = max(1, KC // 4)
                for w in (wq_sb, wk_sb, wv_sb):
                    for q0 in range(0, KC, qn):
                        nc.vector.tensor_tensor(
                            w[:, q0:q0 + qn, :], w[:, q0:q0 + qn, :],
                            lnw_sb[:, q0:q0 + qn, None].to_broadcast(
                                (P, qn, DL)), Alu.mult)

                def phase4(B):
                    # output projection for one gathered 512-query block
                    sl = slice(512 * B, 512 * B + 512)
                    ags = []
                    for kc in range(KC):
                        agt = agpool.tile([P, 512], f8, tag="ag",
                                          name=f"ag{kc}_{B}")
                        eng = nc.sync if kc % 2 == 0 else nc.gpsimd
                        eng.dma_start(agt[:],
                                      agb_out[B][P * kc:P * (kc + 1), :])
                        ags.append(agt)
                    for js in range(DL // P):
                        om = ops.tile([P, 512], f32, tag="om",
                                      name=f"om{js}_{B}")
                        for kc in range(KC):
                            nc.tensor.matmul(
                                om[:], wo_sb[:, kc, P * js:P * (js + 1)],
                                ags[kc][:], start=(kc == 0),
                                stop=(kc == KC - 1))
                        xct = xcpool.tile([P, 512], f32, tag="xct")
                        nc.sync.dma_start(xct[:], xct_in[P * js:P * (js + 1), sl])
                        osb = opool.tile([P, 512], f32, tag="osb")
                        nc.vector.tensor_tensor(osb[:], om[:], xct[:], Alu.add)
                        nc.sync.dma_start(out_cT[P * js:P * (js + 1), sl],
                                          osb[:])

                for B in range(TB):
                    tb = slice(512 * B, 512 * B + 512)
                    # ---------- RMS row sums for this block (off-PE) ------
                    for s in range(4):
                        ts = 4 * B + s
                        if ts not in xr_tiles:
                            t_ = xrpool.tile([P, D], bf16, tag="xr",
                                             name=f"xr{ts}")
                            nc.sync.dma_start(t_[:], xR[P * ts:P * (ts + 1), :])
                            xr_tiles[ts] = t_
                        sq = sqpool.tile([P, D], bf16, tag="sq")
                        nc.scalar.activation(sq[:], xr_tiles[ts][:], Act.Square,
                                             accum_out=ssq_sb[:, ts:ts + 1])
                        del xr_tiles[ts]
                    cs = slice(4 * B, 4 * B + 4)
                    mcol = tmppool.tile([P, 4], f32, tag="mcol")
                    nc.vector.tensor_scalar(mcol[:], ssq_sb[:, cs], 1.0 / D,
                                            EPS, Alu.mult, Alu.add)
                    rrec = tmppool.tile([P, 4], f32, tag="rrec")
                    nc.vector.reciprocal_approx_fast(rrec[:], mcol[:])
                    nc.scalar.activation(rcol_sb[:, cs], rrec[:], Act.Sqrt)
                    for s in range(4):
                        i = 4 * B + s
                        nc.gpsimd.dma_start(
                            out=rrow_sb[0:1, P * i:P * (i + 1)],
                            in_=rcol_sb[:, i:i + 1])
                    rbc = rbcpool.tile([P, 512], f32, tag="rbc")
                    nc.gpsimd.partition_broadcast(rbc[:], rrow_sb[0:1, tb])
                    nc.vector.tensor_tensor(cos_r[:, tb], cos_r[:, tb], rbc[:],
                                            Alu.mult)
                    nc.vector.tensor_tensor(sin_r[:, tb], sin_r[:, tb], rbc[:],
                                            Alu.mult)
                    # ---------- phase 1 for block B ----------
                    qps = qkps.tile([P, NH, 512], f32, tag="qk")
                    for kc in range(KC):
                        for h in range(NH):
                            hs = slice(DH * h, DH * (h + 1))
                            nc.tensor.matmul(qps[:, h, :], wq_sb[:, kc, hs],
                                             xk[kc][:, tb], start=(kc == 0),
                                             stop=(kc == KC - 1))
                    for h in range(NH):
                        nc.vector.tensor_copy(Q_sb[:, h, tb], qps[:, h, :])
                    # K pass (reuses the same psum slot after the Q drain)
                    kps = qkps.tile([P, NH, 512], f32, tag="qk")
                    for kc in range(KC):
                        for h in range(NH):
                            hs = slice(DH * h, DH * (h + 1))
                            nc.tensor.matmul(kps[:, h, :], wk_sb[:, kc, hs],
                                             xk[kc][:, tb], start=(kc == 0),
                                             stop=(kc == KC - 1))
                    for h in range(NH):
                        nc.vector.tensor_copy(K_sb[:, h, tb], kps[:, h, :])
                    # V pass, one 512-row tile (1 psum bank) at a time
                    for ts in range(4):
                        i = 4 * B + ts
                        vp = vps.tile([P, 512], f32, tag="v")
                        for kc in range(KC):
                            nc.tensor.matmul(vp[:, :DL],
                                             xk[kc][:, 512 * B + P * ts:
                                                    512 * B + P * (ts + 1)],
                                             wv_sb[:, kc, :], start=(kc == 0),
                                             stop=(kc == KC - 1))
                        nc.vector.tensor_scalar_mul(V_sb[:, i, :], vp[:, :DL],
                                                    rcol_sb[:, i:i + 1])
                    # RoPE in place on SBUF (r enters via the scaled tables)
                    for buf in (Q_sb, K_sb):
                        for h in range(NH):
                            qs = tmppool.tile([P, 512], bf16, tag="qs")
                            nc.vector.tensor_tensor(qs[:], buf[:, h, tb],
                                                    sin_r[:, tb], Alu.mult)
                            rps = vps.tile([P, 512], f32, tag="v")
                            nc.tensor.matmul(rps[:], rot_sb[:], qs[:],
                                             start=True, stop=True)
                            nc.vector.tensor_tensor(buf[:, h, tb], buf[:, h, tb],
                                                    cos_r[:, tb], Alu.mult)
                            nc.vector.tensor_tensor(buf[:, h, tb], buf[:, h, tb],
                                                    rps[:], Alu.add)
                    # ---------- phase 2 for block B (both heads) ----------
                    ib = tb
                    for h in range(NH):
                        hs = slice(DH * h, DH * (h + 1))
                        av = avpool.tile([P, 512], f32, tag="av")
                        ssum = rowps.tile([1, 512], f32, tag="row")
                        Jmax = 4 * B + 3
                        for J in range(Jmax + 1):
                            st = stpool.tile([P, 512], f32, tag="st")
                            nc.tensor.matmul(st[:],
                                             K_sb[:, h, P * J:P * (J + 1)],
                                             Q_sb[:, h, ib],
                                             start=True, stop=True)
                            if J // 4 == B:
                                nc.vector.tensor_tensor(
                                    st[:], st[:],
                                    masks_sb[:, J % 4, :], Alu.add)
                            pt = ptpool.tile([P, 512], bf16, tag="pt")
                            nc.scalar.activation(pt[:], st[:], Act.Exp,
                                                 scale=inv_sqrt_dh)
                            nc.tensor.matmul(av[:], V_sb[:, J, hs],
                                             pt[:], start=(J == 0),
                                             stop=(J == Jmax))
                            nc.tensor.matmul(ssum[:], ones_bf[:],
                                             pt[:], start=(J == 0),
                                             stop=(J == Jmax))
                        rinv = finpool.tile([1, 512], f32, tag="rinv")
                        nc.vector.reciprocal_approx_fast(rinv[:], ssum[:])
                        rb = finpool.tile([P, 512], f32, tag="rb")
                        nc.gpsimd.partition_broadcast(rb[:], rinv[:])
                        att = finpool.tile([P, 512], f8, tag="att")
                        nc.vector.tensor_tensor(att[:], av[:], rb[:], Alu.mult)
                        nc.sync.dma_start(agb_in[B][DH * h:DH * (h + 1), :],
                                          att[:])
                        if h == NH - 1:
                            nc.gpsimd.collective_compute(
                                "AllGather", Alu.bypass,
                                replica_groups=[list(range(n_cores))],
                                ins=[agb_in[B][:].opt()],
                                outs=[agb_out[B][:].opt()])
                    # ---- phase 4 for the previous block: fills the PE
                    # while this block's collective is in flight
                    if B >= 1:
                        phase4(B - 1)
                phase4(TB - 1)

    nc.compile()
    return nc


# --------------------------------------------------------------------------
# host-side prep / entry point
# --------------------------------------------------------------------------
def prepare_inputs(x, cos, sin, ln_w, Wq, Wk, Wv, Wo, n_cores, heads_per_core):
    import ml_dtypes
    bf16 = ml_dtypes.bfloat16
    DH = 128
    DL = heads_per_core * DH
    x = np.ascontiguousarray(np.asarray(x, dtype=np.float32))
    cos = np.asarray(cos, dtype=np.float32)
    sin = np.asarray(sin, dtype=np.float32)
    ln_w = np.ascontiguousarray(np.asarray(ln_w, dtype=np.float32))
    xT = np.ascontiguousarray(x.T.astype(bf16))
    xR = np.ascontiguousarray(x.astype(bf16))
    cosT = np.ascontiguousarray(cos.T)
    sinT = np.ascontiguousarray(sin.T)
    R = np.zeros((DH, DH), dtype=np.float32)
    R[np.arange(64), np.arange(64) + 64] = -1.0
    R[np.arange(64) + 64, np.arange(64)] = 1.0
    rot_t = np.ascontiguousarray(R.T.astype(bf16))
    # AllGather chunk order: source-core-major, then head; chunk kc = 2c + h
    # holds the 128 att columns (global j = DL*c + DH*h + d) core c / head h
    # contributed.
    perm = np.concatenate([
        DL * cp + DH * h + np.arange(DH)
        for cp in range(n_cores) for h in range(heads_per_core)
    ])
    D = x.shape[1]
    KC = D // DH

    def pretile(wT):
        # (D, DL) -> SBUF layout [P, KC*DL]: element (p, kc, j) = wT[128 kc + p, j]
        return np.ascontiguousarray(
            wT.reshape(KC, DH, DL).transpose(1, 0, 2).reshape(DH, KC * DL)
            .astype(bf16))

    in_maps = []
    for c in range(n_cores):
        cols = slice(c * DL, (c + 1) * DL)
        woT = np.asarray(Wo, np.float32)[cols, :].T  # (D, DL)
        in_maps.append({
            "xT": xT,
            "xR": xR,
            "x_colsT": np.ascontiguousarray(x[:, cols].T),
            "wq_t": pretile(np.asarray(Wq, np.float32)[cols, :].T),
            "wk_t": pretile(np.asarray(Wk, np.float32)[cols, :].T),
            "wv_t": pretile(np.asarray(Wv, np.float32)[cols, :].T),
            "wo_t": pretile(woT[perm, :]),
            "cosT": cosT,
            "sinT": sinT,
            "rot_t": rot_t,
            "ln_w": ln_w,
        })
    return in_maps


_NC_CACHE = {}


def kernel(x, cos, sin, attention_mask, ln_w, Wq, Wk, Wv, Wo,
           _trace=False, _trace_cores=None):
    from concourse.bass_utils import run_bass_kernel_spmd

    cfg = CFG_FULL
    key = tuple(sorted(cfg.items()))
    if key not in _NC_CACHE:
        _NC_CACHE[key] = build_nc(**cfg)
    nc = _NC_CACHE[key]
    n_cores = cfg["n_cores"]
    in_maps = prepare_inputs(x, cos, sin, ln_w, Wq, Wk, Wv, Wo,
                             n_cores, cfg["heads_per_core"])
    res = run_bass_kernel_spmd(nc, in_maps, core_ids=list(range(n_cores)),
                               trace=_trace, trace_cores=_trace_cores)
    out = np.concatenate(
        [res.results[c]["out_colsT"].T for c in range(n_cores)], axis=1)
    kernel.last_result = res
    return out
